# revision 1
# baseline (speedup 1.0000x reference)
"""Two-NEFF Trainium2 kernel for fused BatchNorm1d(train) -> Linear -> ELU.

  y = ELU( ((x - mean) * gamma.rsqrt(var+eps) + beta) @ W.T )

Same algorithm as kernel.py (BN folded into the linear layer), but the 2KB
cross-core stat reduction is done on the HOST between two NEFF launches
instead of with an on-device collective: measured on HW, an
InstCollectiveCompute followed by xbar DMA-transposes costs ~0.5 ms extra
(each transpose serializes against the collective machinery), while the two
NEFFs run at full speed. The 64 MiB/core bf16 staging stays on-device as
sharded jax arrays between the launches.

  NEFF A (per core): stream x (f32), PE ones-matmul stats (bf16 operands),
      downcast x -> bf16, stage feature-split halves to DRAM outputs.
  host: sum the 8x[1,1024] partial stats (32 KB).
  NEFF C (per core): finalize stats -> scaled W.T + bias, xbar-transposed
      reads of staged bf16 x as lhsT, bf16 matmuls, bias via rank-1 matmul,
      ELU = relu(y) + min(exp(y)-1, 0), stream y (f32).
"""

import functools
import sys

import numpy as np

if "/opt/trn_rl_repo" not in sys.path:
    sys.path.insert(0, "/opt/trn_rl_repo")

N_TOTAL = 1048576
F = 256
NCORES = 8
N_SHARD = N_TOTAL // NCORES
P = 128
RT = 8
EPS = 1e-5


def _bass(ncores):
    from concourse import bacc

    return bacc.Bacc(
        "TRN2", target_bir_lowering=False, debug=False, num_devices=ncores
    )


def build_a(n_shard=N_SHARD, ncores=NCORES, rt=RT, repeat=1):
    """Phase A: stats + bf16 staging. Outputs: xb0, xb1 [n_shard,128] bf16,
    st [1, 1024] f32 = [sum(512) | sumsq(512)] (pairs to be folded)."""
    import concourse.tile as tile
    from concourse import mybir

    f32 = mybir.dt.float32
    bf16 = mybir.dt.bfloat16
    AF = mybir.ActivationFunctionType

    nc = _bass(ncores)
    x = nc.dram_tensor("x", [n_shard, F], f32, kind="ExternalInput").ap()
    xb0 = nc.dram_tensor("xb0", [n_shard, P], bf16, kind="ExternalOutput").ap()
    xb1 = nc.dram_tensor("xb1", [n_shard, P], bf16, kind="ExternalOutput").ap()
    st = nc.dram_tensor("st", [1, 4 * F], f32, kind="ExternalOutput").ap()

    T = n_shard // (P * rt)
    n_slices = rt * F // 512

    with tile.TileContext(nc) as tc:
        with tc.tile_pool(name="wp", bufs=1) as wp:
            ones_col = wp.tile([P, 1], bf16)
            nc.vector.memset(ones_col[:], 1.0)
            for _rep in range(repeat):
                with tc.tile_pool(name="sa", bufs=3) as sa, tc.tile_pool(
                    name="psA", bufs=1, space="PSUM"
                ) as psA:
                    ps_sum = psA.tile([1, 512], f32, tag="pssum")
                    ps_sq = psA.tile([1, 512], f32, tag="pssq")
                    xv = x.rearrange("(t p j) f -> t p j f", p=P, j=rt)
                    xb0v = xb0.rearrange("(t p j) c -> t p j c", p=P, j=rt)
                    xb1v = xb1.rearrange("(t p j) c -> t p j c", p=P, j=rt)
                    n_mm = T * n_slices
                    k = 0
                    for t in range(T):
                        xt = sa.tile([P, rt, F], f32, tag="xt")
                        nc.sync.dma_start(xt[:], xv[t])
                        sq = sa.tile([P, rt, F], bf16, tag="sq")
                        nc.scalar.activation(sq[:], xt[:], AF.Square)
                        xb = sa.tile([P, rt, F], bf16, tag="xb")
                        nc.vector.tensor_copy(xb[:], xt[:])
                        nc.sync.dma_start(xb0v[t], xb[:, :, 0:P])
                        nc.sync.dma_start(xb1v[t], xb[:, :, P:F])
                        for j2 in range(n_slices):
                            first = k == 0
                            last = k == n_mm - 1
                            nc.tensor.matmul(
                                ps_sum[:],
                                ones_col[:],
                                xb[:, 2 * j2 : 2 * j2 + 2, :],
                                start=first,
                                stop=last,
                            )
                            nc.tensor.matmul(
                                ps_sq[:],
                                ones_col[:],
                                sq[:, 2 * j2 : 2 * j2 + 2, :],
                                start=first,
                                stop=last,
                            )
                            k += 1
                    stats = wp.tile([1, 4 * F], f32)
                    nc.vector.tensor_copy(stats[:, 0:512], ps_sum[:])
                    nc.vector.tensor_copy(stats[:, 512:1024], ps_sq[:])
                    nc.sync.dma_start(st, stats[:])
    nc.compile()
    return nc


def build_c(n_shard=N_SHARD, n_total=N_TOTAL, ncores=NCORES, rt=RT, repeat=1,
            elu="act"):
    """Phase B'+C: finalize stats (from host-reduced input), matmul + ELU."""
    import concourse.tile as tile
    from concourse import mybir

    f32 = mybir.dt.float32
    bf16 = mybir.dt.bfloat16
    AF = mybir.ActivationFunctionType
    OP = mybir.AluOpType

    nc = _bass(ncores)
    xb0 = nc.dram_tensor("xb0", [n_shard, P], bf16, kind="ExternalInput").ap()
    xb1 = nc.dram_tensor("xb1", [n_shard, P], bf16, kind="ExternalInput").ap()
    wt = nc.dram_tensor("wt", [F, F], f32, kind="ExternalInput").ap()
    gb = nc.dram_tensor("gb", [2, F], f32, kind="ExternalInput").ap()
    sg = nc.dram_tensor("sg", [1, 4 * F], f32, kind="ExternalInput").ap()
    y = nc.dram_tensor("y", [n_shard, F], f32, kind="ExternalOutput").ap()

    T = n_shard // (P * rt)

    with tile.TileContext(nc) as tc:
        with tc.tile_pool(name="wp", bufs=1) as wp, tc.tile_pool(
            name="dram", bufs=1, space="DRAM"
        ) as dr:
            ones_row_bf = wp.tile([1, P], bf16)
            nc.vector.memset(ones_row_bf[:], 1.0)
            wt_sb = wp.tile([P, 2, F], f32)
            nc.sync.dma_start(wt_sb[:], wt.rearrange("(c p) f -> p c f", p=P))
            ga_sb = wp.tile([1, F], f32)
            nc.sync.dma_start(ga_sb[:], gb[0:1, :])
            be_sb = wp.tile([1, F], f32)
            nc.sync.dma_start(be_sb[:], gb[1:2, :])

            for _rep in range(repeat):
                with tc.tile_pool(name="pb", bufs=1, space="PSUM") as psB:
                    g = wp.tile([1, 4 * F], f32)
                    nc.sync.dma_start(g[:], sg)
                    fsum = wp.tile([1, F], f32)
                    nc.vector.tensor_add(fsum[:], g[:, 0:F], g[:, F : 2 * F])
                    fsq = wp.tile([1, F], f32)
                    nc.vector.tensor_add(
                        fsq[:], g[:, 2 * F : 3 * F], g[:, 3 * F : 4 * F]
                    )
                    mean = wp.tile([1, F], f32)
                    nc.vector.tensor_scalar_mul(mean[:], fsum[:], 1.0 / n_total)
                    var = wp.tile([1, F], f32)
                    nc.vector.tensor_mul(var[:], mean[:], mean[:])
                    nc.vector.scalar_tensor_tensor(
                        var[:], fsq[:], 1.0 / n_total, var[:], OP.mult, OP.subtract
                    )
                    nc.vector.tensor_scalar_add(var[:], var[:], EPS)
                    inv = wp.tile([1, F], f32)
                    nc.vector.reciprocal(inv[:], var[:])
                    rstd = wp.tile([1, F], f32)
                    nc.scalar.activation(rstd[:], inv[:], AF.Sqrt)
                    srow = wp.tile([1, F], f32)
                    nc.vector.tensor_mul(srow[:], ga_sb[:], rstd[:])
                    trow = wp.tile([1, F], f32)
                    nc.vector.tensor_mul(trow[:], mean[:], srow[:])
                    nc.vector.tensor_sub(trow[:], be_sb[:], trow[:])

                    st_d = dr.tile([2, F], f32)
                    nc.sync.dma_start(st_d[0:1, :], srow[:])
                    nc.sync.dma_start(st_d[1:2, :], trow[:])
                    sT = wp.tile([P, 2], f32)
                    nc.sync.dma_start(
                        sT[:], st_d[0:1, :].rearrange("o (c p) -> p (o c)", p=P)
                    )
                    tT = wp.tile([P, 2], f32)
                    nc.sync.dma_start(
                        tT[:], st_d[1:2, :].rearrange("o (c p) -> p (o c)", p=P)
                    )
                    wts = wp.tile([P, 2, F], bf16)
                    for c in range(2):
                        nc.vector.tensor_scalar(
                            wts[:, c, :],
                            wt_sb[:, c, :],
                            sT[:, c : c + 1],
                            None,
                            OP.mult,
                        )
                    ps_b = psB.tile([1, F], f32, tag="psb")
                    for c in range(2):
                        nc.tensor.matmul(
                            ps_b[:],
                            tT[:, c : c + 1],
                            wt_sb[:, c, :],
                            start=(c == 0),
                            stop=(c == 1),
                        )
                    b_bf = wp.tile([1, F], bf16)
                    nc.vector.tensor_copy(b_bf[:], ps_b[:])

                with tc.tile_pool(name="cp", bufs=3) as cp, tc.tile_pool(
                    name="psC", bufs=2, space="PSUM"
                ) as psC:
                    yv = y.rearrange("(t rb q) f -> t q rb f", q=P, rb=rt)
                    R = P * rt
                    for t in range(T):
                        xT0 = cp.tile([P, R], bf16, tag="x0")
                        nc.sync.dma_start_transpose(
                            xT0[:], xb0[t * R : (t + 1) * R, :]
                        )
                        xT1 = cp.tile([P, R], bf16, tag="x1")
                        nc.sync.dma_start_transpose(
                            xT1[:], xb1[t * R : (t + 1) * R, :]
                        )
                        ps_y = psC.tile([P, rt * F], f32, tag="psy")
                        for rb in range(rt):
                            sl = slice(rb * P, (rb + 1) * P)
                            out_sl = ps_y[:, rb * F : (rb + 1) * F]
                            nc.tensor.matmul(
                                out_sl, xT0[:, sl], wts[:, 0, :], start=True, stop=False
                            )
                            nc.tensor.matmul(
                                out_sl, xT1[:, sl], wts[:, 1, :], start=False, stop=False
                            )
                            nc.tensor.matmul(
                                out_sl, ones_row_bf[:], b_bf[:], start=False, stop=True
                            )
                        e = cp.tile([P, rt * F], f32, tag="e")
                        nc.scalar.activation(e[:], ps_y[:], AF.Exp)
                        yo = cp.tile([P, rt * F], f32, tag="yo")
                        if elu == "dve":
                            # em = min(e-1, 0); yo = max(psy, 0) + em (DVE-heavy)
                            em = cp.tile([P, rt * F], f32, tag="em")
                            nc.vector.tensor_scalar(
                                em[:], e[:], 1.0, 0.0, OP.subtract, OP.min
                            )
                            nc.vector.scalar_tensor_tensor(
                                yo[:], ps_y[:], 0.0, em[:], OP.max, OP.add
                            )
                        else:
                            # ELU(v) = min(exp(v)-1, relu(v)) (ACT-heavy)
                            r = cp.tile([P, rt * F], f32, tag="r")
                            nc.scalar.activation(r[:], ps_y[:], AF.Relu)
                            nc.vector.scalar_tensor_tensor(
                                yo[:], e[:], 1.0, r[:], OP.subtract, OP.min
                            )
                        nc.sync.dma_start(
                            yv[t], yo[:].rearrange("q (rb f) -> q rb f", f=F)
                        )
    nc.compile()
    return nc


@functools.lru_cache(maxsize=4)
def _built_a(repeat=1):
    return build_a(repeat=repeat)


@functools.lru_cache(maxsize=4)
def _built_c(repeat=1, elu="act"):
    return build_c(repeat=repeat, elu=elu)


def _pjrt_fn(nc, ncores=NCORES):
    """Compile a bass module into a jitted 8-core shard_map callable.
    Returns (fn, in_names, out_names, out_avals)."""
    import jax
    from jax.experimental.shard_map import shard_map
    from jax.sharding import Mesh, PartitionSpec

    from concourse import mybir
    from concourse.bass2jax import (
        _bass_exec_p,
        install_neuronx_cc_hook,
        partition_id_tensor,
    )

    install_neuronx_cc_hook()
    partition_name = nc.partition_id_tensor.name if nc.partition_id_tensor else None
    in_names, out_names, out_avals = [], [], []
    for alloc in nc.m.functions[0].allocations:
        if not isinstance(alloc, mybir.MemoryLocationSet):
            continue
        name = alloc.memorylocations[0].name
        if alloc.kind == "ExternalInput":
            if name != partition_name:
                in_names.append(name)
        elif alloc.kind == "ExternalOutput":
            out_names.append(name)
            out_avals.append(
                jax.core.ShapedArray(
                    tuple(alloc.tensor_shape), mybir.dt.np(alloc.dtype)
                )
            )
    n_params = len(in_names)
    all_in_names = list(in_names) + list(out_names)
    if partition_name is not None:
        all_in_names.append(partition_name)

    def _body(*args):
        operands = list(args)
        if partition_name is not None:
            operands.append(partition_id_tensor())
        outs = _bass_exec_p.bind(
            *operands,
            out_avals=tuple(out_avals),
            in_names=tuple(all_in_names),
            out_names=tuple(out_names),
            lowering_input_output_aliases=(),
            sim_require_finite=True,
            sim_require_nnan=True,
            nc=nc,
        )
        return tuple(outs)

    devices = jax.devices()[:ncores]
    mesh = Mesh(np.asarray(devices), ("core",))
    spec = PartitionSpec("core")
    fn = jax.jit(
        shard_map(
            _body,
            mesh=mesh,
            in_specs=(spec,) * (n_params + len(out_names)),
            out_specs=(spec,) * len(out_names),
            check_rep=False,
        ),
        keep_unused=True,
    )
    return fn, in_names, out_names, out_avals, mesh


def _sharding():
    import jax
    from jax.sharding import Mesh, NamedSharding, PartitionSpec

    devices = jax.devices()[:NCORES]
    mesh = Mesh(np.asarray(devices), ("core",))
    return NamedSharding(mesh, PartitionSpec("core"))


def _zeros_for(out_avals):
    return [
        np.zeros((NCORES * av.shape[0], *av.shape[1:]), av.dtype) for av in out_avals
    ]


def kernel(x, gamma, beta, W):
    import jax

    x = np.ascontiguousarray(np.asarray(x), dtype=np.float32)
    gamma = np.asarray(gamma, dtype=np.float32)
    beta = np.asarray(beta, dtype=np.float32)
    W = np.asarray(W, dtype=np.float32)
    assert x.shape == (N_TOTAL, F), x.shape

    sharding = _sharding()

    # ---- NEFF A: stats + staging
    nc_a = _built_a()
    fn_a, in_a, out_a, av_a, _ = _pjrt_fn(nc_a)
    assert in_a == ["x"], in_a
    x_dev = jax.device_put(x, sharding)
    outs_a = fn_a(x_dev, *[jax.device_put(z, sharding) for z in _zeros_for(av_a)])
    outs_a = dict(zip(out_a, outs_a))

    # ---- host: reduce the 8 partial stat rows (32 KB total)
    st_host = np.asarray(outs_a["st"])  # [8*1, 1024]
    stats_g = st_host.reshape(NCORES, 4 * F).sum(axis=0, dtype=np.float64)
    stats_g = np.ascontiguousarray(
        np.broadcast_to(stats_g.astype(np.float32), (NCORES, 4 * F))
    )

    # ---- NEFF C: matmul + ELU (staging stays on device)
    nc_c = _built_c()
    fn_c, in_c, out_c, av_c, _ = _pjrt_fn(nc_c)
    wt_host = np.ascontiguousarray(W.T)
    gb_host = np.stack([gamma, beta])
    host_ins = {
        "wt": np.concatenate([wt_host] * NCORES, axis=0),
        "gb": np.concatenate([gb_host] * NCORES, axis=0),
        "sg": stats_g,
    }
    args_c = []
    for nm in in_c:
        if nm in ("xb0", "xb1"):
            args_c.append(outs_a[nm])
        else:
            args_c.append(jax.device_put(host_ins[nm], sharding))
    outs_c = fn_c(*args_c, *[jax.device_put(z, sharding) for z in _zeros_for(av_c)])
    y = np.asarray(outs_c[out_c.index("y")])
    return np.ascontiguousarray(y.reshape(N_TOTAL, F), dtype=np.float32)


if __name__ == "__main__":
    nca = build_a()
    ncc = build_c()
    print("built OK")



# revision 4
# speedup vs baseline: 1.5811x; 1.5811x over previous
"""Two-NEFF Trainium2 kernel for fused BatchNorm1d(train) -> Linear -> ELU.

  y = ELU( ((x - mean) * gamma.rsqrt(var+eps) + beta) @ W.T )

Data-parallel over 8 cores (rows sharded). BN stats are reduced on the HOST
between two NEFF launches (a 4 KB exchange; an on-device collective measured
~0.5 ms slower in a previous session).

Key layout decision vs the earlier baseline: the earlier kernel staged x in
ROW-major bf16 and phase C re-read it with `dma_start_transpose`, which
measured ~106 GB/s effective (xbar-transpose DMA serializes badly) and made
phase C 1.26 ms. Here phase A transposes ON-CHIP with the PE (identity
matmul) and stages x TRANSPOSED, so phase C does only plain contiguous DMA.

  NEFF A (per core): stream x tiles (bf16, host pre-cast), PE-transpose
      16x [128,128] blocks per tile into PSUM, ACT-copy to bf16 SBUF
      (accum_out gives per-feature sums for free), DVE square-reduce gives
      per-feature sum-of-squares; stage transposed tiles to DRAM.
      Stats out: st [128, 4] = (sum_h0, sum_h1, ssq_h0, ssq_h1).
  host: sum the 8 st tiles, finalize scale/shift s,t in f64:
      s = gamma * rsqrt(var+eps), t = beta - mean*s.
  NEFF C (per core): preamble folds s into W.T (bf16) and computes the
      bias row b = t @ W.T with two rank-128 matmuls; main loop reads
      staged xT tiles (contiguous), 3 matmuls per 128-row group
      (two f_in halves + rank-1 bias), ELU = min(exp(y)-1, relu(y)),
      writes y in bf16 (upcast to f32 on host).

Row mapping: x rows are loaded as [t, p, j] (row = t*1024 + p*8 + j), the
j-th block-column of the transposed tile holds rows {c*8+j}; after the
matmul, PSUM partition q of group j is row q*8+j, so y tiles write back
8 CONSECUTIVE rows per partition (4 KiB contiguous descriptors).
"""

import functools
import sys

import numpy as np

if "/opt/trn_rl_repo" not in sys.path:
    sys.path.insert(0, "/opt/trn_rl_repo")

N_TOTAL = 1048576
F = 256
NCORES = 8
N_SHARD = N_TOTAL // NCORES
P = 128
RT = 8
T = N_SHARD // (P * RT)
EPS = 1e-5


def _bass(ncores):
    from concourse import bacc

    return bacc.Bacc(
        "TRN2", target_bir_lowering=False, debug=False, num_devices=ncores
    )


def build_a(n_shard=N_SHARD, ncores=NCORES, repeat=1):
    """Phase A: on-chip transpose to bf16 staging + BN partial stats.

    Inputs: x [n_shard, 256] bf16, ident [128, 128] bf16.
    Outputs: xt [(T*2*128), 1024] bf16 (transposed staging),
             st [128, 4] f32 = (sum_h0, sum_h1, ssq_h0, ssq_h1).
    """
    import concourse.tile as tile
    from concourse import mybir

    f32 = mybir.dt.float32
    bf16 = mybir.dt.bfloat16
    AF = mybir.ActivationFunctionType
    OP = mybir.AluOpType
    AX = mybir.AxisListType

    t_count = n_shard // (P * RT)

    nc = _bass(ncores)
    x = nc.dram_tensor("x", [n_shard, F], bf16, kind="ExternalInput").ap()
    ident = nc.dram_tensor("ident", [P, P], bf16, kind="ExternalInput").ap()
    xt = nc.dram_tensor(
        "xt", [t_count * 2 * P, RT * P], bf16, kind="ExternalOutput"
    ).ap()
    st = nc.dram_tensor("st", [P, 4], f32, kind="ExternalOutput").ap()

    with tile.TileContext(nc) as tc:
        with tc.tile_pool(name="wp", bufs=1) as wp:
            id_sb = wp.tile([P, P], bf16)
            nc.sync.dma_start(id_sb[:], ident)
            for _rep in range(repeat):
                with tc.tile_pool(name="sa", bufs=3) as sa, tc.tile_pool(
                    name="sbp", bufs=1
                ) as sbp, tc.tile_pool(name="psA", bufs=2, space="PSUM") as psA:
                    sum_buf = sbp.tile([P, 2, t_count], f32)
                    ssq_buf = sbp.tile([P, 2, t_count], f32)
                    xv = x.rearrange("(t p j) f -> t p j f", p=P, j=RT)
                    xtv = xt.rearrange("(t h p) c -> t h p c", h=2, p=P)
                    for t in range(t_count):
                        xin = sa.tile([P, RT, F], bf16, tag="xin")
                        nc.sync.dma_start(xin[:], xv[t])
                        ps = psA.tile([P, 2, RT * P], bf16, tag="ps")
                        for h in range(2):
                            for j in range(RT):
                                nc.tensor.transpose(
                                    ps[:, h, j * P : (j + 1) * P],
                                    xin[:, j, h * P : (h + 1) * P],
                                    id_sb[:],
                                )
                        for h in range(2):
                            xth = sa.tile([P, RT * P], bf16, tag=f"xt{h}")
                            # PSUM f32 -> SBUF bf16; accum_out = per-feature sum
                            nc.scalar.activation(
                                xth[:],
                                ps[:, h],
                                AF.Identity,
                                accum_out=sum_buf[:, h, t : t + 1],
                            )
                            nc.sync.dma_start(xtv[t, h], xth[:])
                            # DVE accum_out crashes on HW here; square via
                            # plain tensor_tensor, then free-axis reduce.
                            scr = sa.tile([P, RT * P], bf16, tag=f"scr{h}")
                            nc.vector.tensor_tensor(
                                scr[:], xth[:], xth[:], OP.mult
                            )
                            nc.vector.tensor_reduce(
                                ssq_buf[:, h, t : t + 1], scr[:], AX.X, OP.add
                            )
                    stv = sbp.tile([P, 4], f32)
                    for h in range(2):
                        nc.vector.tensor_reduce(
                            stv[:, h : h + 1], sum_buf[:, h], AX.X, OP.add
                        )
                        nc.vector.tensor_reduce(
                            stv[:, 2 + h : 3 + h], ssq_buf[:, h], AX.X, OP.add
                        )
                    nc.sync.dma_start(st, stv[:])
    nc.compile()
    return nc


def build_c(n_shard=N_SHARD, ncores=NCORES, repeat=1):
    """Phase C: matmul from transposed staging + ELU, bf16 output.

    Inputs: xt [(T*2*128), 1024] bf16, wt [256, 256] f32 (= W.T),
            aff [128, 4] f32 = (s_h0, s_h1, t_h0, t_h1).
    Output: y [n_shard, 256] bf16.
    """
    import concourse.tile as tile
    from concourse import mybir

    f32 = mybir.dt.float32
    bf16 = mybir.dt.bfloat16
    AF = mybir.ActivationFunctionType
    OP = mybir.AluOpType

    t_count = n_shard // (P * RT)

    nc = _bass(ncores)
    xt = nc.dram_tensor(
        "xt", [t_count * 2 * P, RT * P], bf16, kind="ExternalInput"
    ).ap()
    wt = nc.dram_tensor("wt", [F, F], f32, kind="ExternalInput").ap()
    aff = nc.dram_tensor("aff", [P, 4], f32, kind="ExternalInput").ap()
    y = nc.dram_tensor("y", [n_shard, F], bf16, kind="ExternalOutput").ap()

    with tile.TileContext(nc) as tc:
        with tc.tile_pool(name="wp", bufs=1) as wp:
            ones_row = wp.tile([1, P], bf16)
            nc.vector.memset(ones_row[:], 1.0)
            for _rep in range(repeat):
                with tc.tile_pool(name="pre", bufs=1) as pre, tc.tile_pool(
                    name="psB", bufs=1, space="PSUM"
                ) as psB:
                    wt_sb = pre.tile([P, 2, F], f32)
                    nc.sync.dma_start(
                        wt_sb[:], wt.rearrange("(h p) f -> p h f", p=P)
                    )
                    aff_sb = pre.tile([P, 4], f32)
                    nc.sync.dma_start(aff_sb[:], aff)
                    # ws[h] = W.T[h-half] * s[h] (bf16), wb[h] = raw W.T bf16
                    ws = wp.tile([P, 2, F], bf16)
                    wb = pre.tile([P, 2, F], bf16)
                    tcol = pre.tile([P, 2], bf16)
                    nc.vector.tensor_copy(tcol[:], aff_sb[:, 2:4])
                    for h in range(2):
                        nc.vector.tensor_scalar(
                            ws[:, h],
                            wt_sb[:, h],
                            aff_sb[:, h : h + 1],
                            None,
                            OP.mult,
                        )
                        nc.vector.tensor_copy(wb[:, h], wt_sb[:, h])
                    # bias row b = t @ W.T  (two K=128 matmuls)
                    ps_b = psB.tile([1, F], f32)
                    for h in range(2):
                        nc.tensor.matmul(
                            ps_b[:],
                            tcol[:, h : h + 1],
                            wb[:, h],
                            start=(h == 0),
                            stop=(h == 1),
                        )
                    b_bf = wp.tile([1, F], bf16)
                    nc.vector.tensor_copy(b_bf[:], ps_b[:])

                with tc.tile_pool(name="cp", bufs=3) as cp, tc.tile_pool(
                    name="psC", bufs=2, space="PSUM"
                ) as psC:
                    xtv = xt.rearrange("(t h p) c -> t h p c", h=2, p=P)
                    yv = y.rearrange("(t p j) f -> t p j f", p=P, j=RT)
                    for t in range(t_count):
                        xt0 = cp.tile([P, RT * P], bf16, tag="x0")
                        nc.sync.dma_start(xt0[:], xtv[t, 0])
                        xt1 = cp.tile([P, RT * P], bf16, tag="x1")
                        nc.sync.dma_start(xt1[:], xtv[t, 1])
                        ps_y = psC.tile([P, RT, F], f32, tag="psy")
                        for j in range(RT):
                            sl = slice(j * P, (j + 1) * P)
                            nc.tensor.matmul(
                                ps_y[:, j],
                                xt0[:, sl],
                                ws[:, 0],
                                start=True,
                                stop=False,
                            )
                            nc.tensor.matmul(
                                ps_y[:, j],
                                xt1[:, sl],
                                ws[:, 1],
                                start=False,
                                stop=False,
                            )
                            nc.tensor.matmul(
                                ps_y[:, j],
                                ones_row[:],
                                b_bf[:],
                                start=False,
                                stop=True,
                            )
                        # ELU(v) = min(exp(v)-1, relu(v))
                        e = cp.tile([P, RT * F], bf16, tag="e")
                        nc.scalar.activation(e[:], ps_y[:], AF.Exp)
                        r = cp.tile([P, RT * F], bf16, tag="r")
                        nc.scalar.activation(r[:], ps_y[:], AF.Relu)
                        yo = cp.tile([P, RT * F], bf16, tag="yo")
                        nc.vector.scalar_tensor_tensor(
                            yo[:], e[:], 1.0, r[:], OP.subtract, OP.min
                        )
                        nc.sync.dma_start(
                            yv[t], yo[:].rearrange("q (j f) -> q j f", f=F)
                        )
    nc.compile()
    return nc


@functools.lru_cache(maxsize=4)
def _built_a(repeat=1):
    return build_a(repeat=repeat)


@functools.lru_cache(maxsize=4)
def _built_c(repeat=1):
    return build_c(repeat=repeat)


def _pjrt_fn(nc, ncores=NCORES):
    """Compile a bass module into a jitted 8-core shard_map callable.
    Returns (fn, in_names, out_names, out_avals, mesh)."""
    import jax
    from jax.experimental.shard_map import shard_map
    from jax.sharding import Mesh, PartitionSpec

    from concourse import mybir
    from concourse.bass2jax import (
        _bass_exec_p,
        install_neuronx_cc_hook,
        partition_id_tensor,
    )

    install_neuronx_cc_hook()
    partition_name = nc.partition_id_tensor.name if nc.partition_id_tensor else None
    in_names, out_names, out_avals = [], [], []
    for alloc in nc.m.functions[0].allocations:
        if not isinstance(alloc, mybir.MemoryLocationSet):
            continue
        name = alloc.memorylocations[0].name
        if alloc.kind == "ExternalInput":
            if name != partition_name:
                in_names.append(name)
        elif alloc.kind == "ExternalOutput":
            out_names.append(name)
            out_avals.append(
                jax.core.ShapedArray(
                    tuple(alloc.tensor_shape), mybir.dt.np(alloc.dtype)
                )
            )
    n_params = len(in_names)
    all_in_names = list(in_names) + list(out_names)
    if partition_name is not None:
        all_in_names.append(partition_name)

    def _body(*args):
        operands = list(args)
        if partition_name is not None:
            operands.append(partition_id_tensor())
        outs = _bass_exec_p.bind(
            *operands,
            out_avals=tuple(out_avals),
            in_names=tuple(all_in_names),
            out_names=tuple(out_names),
            lowering_input_output_aliases=(),
            sim_require_finite=True,
            sim_require_nnan=True,
            nc=nc,
        )
        return tuple(outs)

    devices = jax.devices()[:ncores]
    mesh = Mesh(np.asarray(devices), ("core",))
    spec = PartitionSpec("core")
    fn = jax.jit(
        shard_map(
            _body,
            mesh=mesh,
            in_specs=(spec,) * (n_params + len(out_names)),
            out_specs=(spec,) * len(out_names),
            check_rep=False,
        ),
        keep_unused=True,
    )
    return fn, in_names, out_names, out_avals, mesh


def _sharding():
    import jax
    from jax.sharding import Mesh, NamedSharding, PartitionSpec

    devices = jax.devices()[:NCORES]
    mesh = Mesh(np.asarray(devices), ("core",))
    return NamedSharding(mesh, PartitionSpec("core"))


def _zeros_for(out_avals):
    return [
        np.zeros((NCORES * av.shape[0], *av.shape[1:]), av.dtype) for av in out_avals
    ]


def kernel(x, gamma, beta, W):
    import jax
    import jax.numpy as jnp

    gamma = np.asarray(gamma, dtype=np.float64)
    beta = np.asarray(beta, dtype=np.float64)
    W = np.asarray(W, dtype=np.float32)
    assert np.asarray(x).shape == (N_TOTAL, F)

    cpu = jax.devices("cpu")[0]
    with jax.default_device(cpu):
        x_bf = np.asarray(jnp.asarray(np.asarray(x)).astype(jnp.bfloat16))

    sharding = _sharding()

    # ---- NEFF A: on-chip transpose + staging + partial stats
    nc_a = _built_a()
    fn_a, in_a, out_a, av_a, _ = _pjrt_fn(nc_a)
    ident = np.concatenate([np.eye(P, dtype=x_bf.dtype)] * NCORES, axis=0)
    host_a = {"x": x_bf, "ident": ident}
    args_a = [jax.device_put(host_a[nm], sharding) for nm in in_a]
    outs_a = fn_a(*args_a, *[jax.device_put(z, sharding) for z in _zeros_for(av_a)])
    outs_a = dict(zip(out_a, outs_a))

    # ---- host: reduce the 8 partial stat tiles (16 KB), finalize scale/shift
    st_host = np.asarray(outs_a["st"]).astype(np.float64)  # [8*128, 4]
    st_sum = st_host.reshape(NCORES, P, 4).sum(axis=0)  # [128, 4]
    mean = st_sum[:, 0:2] / N_TOTAL  # [128, 2] (h columns)
    var = st_sum[:, 2:4] / N_TOTAL - mean**2
    g_cols = np.stack([gamma[0:P], gamma[P:F]], axis=1)
    b_cols = np.stack([beta[0:P], beta[P:F]], axis=1)
    s_cols = g_cols / np.sqrt(var + EPS)
    t_cols = b_cols - mean * s_cols
    aff = np.concatenate([s_cols, t_cols], axis=1).astype(np.float32)  # [128,4]

    # ---- NEFF C: matmul + ELU (staging stays on device)
    nc_c = _built_c()
    fn_c, in_c, out_c, av_c, _ = _pjrt_fn(nc_c)
    host_c = {
        "wt": np.concatenate([np.ascontiguousarray(W.T)] * NCORES, axis=0),
        "aff": np.concatenate([aff] * NCORES, axis=0),
    }
    args_c = []
    for nm in in_c:
        if nm == "xt":
            args_c.append(outs_a["xt"])
        else:
            args_c.append(jax.device_put(host_c[nm], sharding))
    outs_c = fn_c(*args_c, *[jax.device_put(z, sharding) for z in _zeros_for(av_c)])
    y_bf = np.asarray(outs_c[out_c.index("y")])
    with jax.default_device(cpu):
        y = np.asarray(jnp.asarray(y_bf).astype(jnp.float32))
    return np.ascontiguousarray(y.reshape(N_TOTAL, F))


if __name__ == "__main__":
    nca = build_a()
    ncc = build_c()
    print("built OK")


# revision 8
# speedup vs baseline: 2.9074x; 1.8389x over previous
"""Two-NEFF Trainium2 kernel for fused BatchNorm1d(train) -> Linear -> ELU.

  y = ELU( ((x - mean) * gamma.rsqrt(var+eps) + beta) @ W.T )

Data-parallel over 8 cores (rows sharded). BN stats are reduced on the HOST
between two NEFF launches (a 4 KB exchange; an on-device collective measured
~0.5 ms slower in a previous session).

Key layout decision vs the earlier baseline: the earlier kernel staged x in
ROW-major bf16 and phase C re-read it with `dma_start_transpose`, which
measured ~106 GB/s effective (xbar-transpose DMA serializes badly) and made
phase C 1.26 ms. Here phase A transposes ON-CHIP with the PE (identity
matmul) and stages x TRANSPOSED, so phase C does only plain contiguous DMA.

  NEFF A (per core): stream x tiles (bf16, host pre-cast), PE-transpose
      16x [128,128] blocks per tile into PSUM, ACT-copy to bf16 SBUF
      (accum_out gives per-feature sums for free), DVE square-reduce gives
      per-feature sum-of-squares; stage transposed tiles to DRAM.
      Stats out: st [128, 4] = (sum_h0, sum_h1, ssq_h0, ssq_h1).
  host: sum the 8 st tiles, finalize scale/shift s,t in f64:
      s = gamma * rsqrt(var+eps), t = beta - mean*s.
  NEFF C (per core): preamble folds s into W.T (bf16) and computes the
      bias row b = t @ W.T with two rank-128 matmuls; main loop reads
      staged xT tiles (contiguous), 3 matmuls per 128-row group
      (two f_in halves + rank-1 bias), ELU = min(exp(y)-1, relu(y)),
      writes y in bf16 (upcast to f32 on host).

Row mapping: x rows are loaded as [t, p, j] (row = t*1024 + p*8 + j), the
j-th block-column of the transposed tile holds rows {c*8+j}; after the
matmul, PSUM partition q of group j is row q*8+j, so y tiles write back
8 CONSECUTIVE rows per partition (4 KiB contiguous descriptors).
"""

import functools
import sys

import numpy as np

if "/opt/trn_rl_repo" not in sys.path:
    sys.path.insert(0, "/opt/trn_rl_repo")

N_TOTAL = 1048576
F = 256
NCORES = 8
N_SHARD = N_TOTAL // NCORES
P = 128
RT = 8
T = N_SHARD // (P * RT)
EPS = 1e-5


def _bass(ncores):
    from concourse import bacc

    return bacc.Bacc(
        "TRN2", target_bir_lowering=False, debug=False, num_devices=ncores
    )


def build_a(n_shard=N_SHARD, ncores=NCORES, repeat=1):
    """Phase A: on-chip transpose to bf16 staging + BN partial stats.

    Inputs: x [n_shard, 256] bf16, ident [128, 128] bf16.
    Outputs: xt [(T*2*128), 1024] bf16 (transposed staging),
             st [128, 4] f32 = (sum_h0, sum_h1, ssq_h0, ssq_h1).
    """
    import concourse.tile as tile
    from concourse import mybir

    f32 = mybir.dt.float32
    bf16 = mybir.dt.bfloat16
    AF = mybir.ActivationFunctionType
    OP = mybir.AluOpType
    AX = mybir.AxisListType

    t_count = n_shard // (P * RT)

    nc = _bass(ncores)
    x = nc.dram_tensor("x", [n_shard, F], bf16, kind="ExternalInput").ap()
    ident = nc.dram_tensor("ident", [P, P], bf16, kind="ExternalInput").ap()
    xt = nc.dram_tensor(
        "xt", [t_count * 2 * P, RT * P], bf16, kind="ExternalOutput"
    ).ap()
    st = nc.dram_tensor("st", [P, 4], f32, kind="ExternalOutput").ap()

    with tile.TileContext(nc) as tc:
        with tc.tile_pool(name="wp", bufs=1) as wp:
            id_sb = wp.tile([P, P], bf16)
            nc.sync.dma_start(id_sb[:], ident)
            for _rep in range(repeat):
                with tc.tile_pool(name="sa", bufs=3) as sa, tc.tile_pool(
                    name="sbp", bufs=1
                ) as sbp, tc.tile_pool(name="psA", bufs=2, space="PSUM") as psA:
                    sum_buf = sbp.tile([P, 2, t_count], f32)
                    ssq_buf = sbp.tile([P, 2, t_count], f32)
                    xv = x.rearrange("(t p j) f -> t p j f", p=P, j=RT)
                    xtv = xt.rearrange("(t h p) c -> t h p c", h=2, p=P)
                    for t in range(t_count):
                        xin = sa.tile([P, RT, F], bf16, tag="xin")
                        nc.sync.dma_start(xin[:], xv[t])
                        ps = psA.tile([P, 2, RT * P], bf16, tag="ps")
                        for h in range(2):
                            for j in range(RT):
                                nc.tensor.transpose(
                                    ps[:, h, j * P : (j + 1) * P],
                                    xin[:, j, h * P : (h + 1) * P],
                                    id_sb[:],
                                )
                        for h in range(2):
                            xth = sa.tile([P, RT * P], bf16, tag=f"xt{h}")
                            # PSUM f32 -> SBUF bf16; accum_out = per-feature sum
                            nc.scalar.activation(
                                xth[:],
                                ps[:, h],
                                AF.Identity,
                                accum_out=sum_buf[:, h, t : t + 1],
                            )
                            nc.sync.dma_start(xtv[t, h], xth[:])
                            # DVE accum_out crashes on HW here; square via
                            # plain tensor_tensor, then free-axis reduce.
                            scr = sa.tile([P, RT * P], bf16, tag=f"scr{h}")
                            nc.vector.tensor_tensor(
                                scr[:], xth[:], xth[:], OP.mult
                            )
                            nc.vector.tensor_reduce(
                                ssq_buf[:, h, t : t + 1], scr[:], AX.X, OP.add
                            )
                    stv = sbp.tile([P, 4], f32)
                    for h in range(2):
                        nc.vector.tensor_reduce(
                            stv[:, h : h + 1], sum_buf[:, h], AX.X, OP.add
                        )
                        nc.vector.tensor_reduce(
                            stv[:, 2 + h : 3 + h], ssq_buf[:, h], AX.X, OP.add
                        )
                    nc.sync.dma_start(st, stv[:])
    nc.compile()
    return nc


def build_c(n_shard=N_SHARD, ncores=NCORES, repeat=1):
    """Phase C: matmul from transposed staging + ELU, TRANSPOSED bf16 output.

    Computes yT = (s*W.T).T-blocks @ xT + b so the small W blocks are the
    PE-stationary operand (4 reused loads per tile instead of 16) and the
    linear bias b = t @ W.T is PER-PARTITION, riding the ACT/DVE ops for
    free. The host un-transposes the blocked output.

    Inputs: xt [(T*2*128), 1024] bf16, wt [256, 256] f32 (= W.T),
            aff [128, 4] f32 = (s_h0, s_h1, b_q0, b_q1).
    Output: yt [(T*2*128), 1024] bf16, blocked [t, q, p_fout, (j, c)]
            = y[row t*1024 + c*8 + j, fout q*128 + p_fout].
    """
    import concourse.tile as tile
    from concourse import mybir

    f32 = mybir.dt.float32
    bf16 = mybir.dt.bfloat16
    AF = mybir.ActivationFunctionType
    OP = mybir.AluOpType

    t_count = n_shard // (P * RT)
    NB = RT * P // 2  # 512: psum-bank-sized matmul N

    nc = _bass(ncores)
    xt = nc.dram_tensor(
        "xt", [t_count * 2 * P, RT * P], bf16, kind="ExternalInput"
    ).ap()
    wt = nc.dram_tensor("wt", [F, F], f32, kind="ExternalInput").ap()
    aff = nc.dram_tensor("aff", [P, 4], f32, kind="ExternalInput").ap()
    yt = nc.dram_tensor(
        "yt", [t_count * 2 * P, RT * P], bf16, kind="ExternalOutput"
    ).ap()

    with tile.TileContext(nc) as tc:
        with tc.tile_pool(name="wp", bufs=1) as wp:
            for _rep in range(repeat):
                with tc.tile_pool(name="pre", bufs=1) as pre:
                    wt_sb = pre.tile([P, 2, F], f32)
                    nc.sync.dma_start(
                        wt_sb[:], wt.rearrange("(h p) f -> p h f", p=P)
                    )
                    aff_sb = wp.tile([P, 4], f32)
                    nc.sync.dma_start(aff_sb[:], aff)
                    # ws[h] = W.T[h-half] * s[h] (bf16)
                    ws = wp.tile([P, 2, F], bf16)
                    for h in range(2):
                        nc.vector.tensor_scalar(
                            ws[:, h],
                            wt_sb[:, h],
                            aff_sb[:, h : h + 1],
                            None,
                            OP.mult,
                        )

                with tc.tile_pool(name="cp", bufs=3) as cp, tc.tile_pool(
                    name="psC", bufs=2, space="PSUM"
                ) as psC:
                    xtv = xt.rearrange("(t h p) c -> t h p c", h=2, p=P)
                    ytv = yt.rearrange("(t q p) c -> t q p c", q=2, p=P)
                    for t in range(t_count):
                        xt0 = cp.tile([P, RT * P], bf16, tag="x0")
                        nc.sync.dma_start(xt0[:], xtv[t, 0])
                        xt1 = cp.tile([P, RT * P], bf16, tag="x1")
                        nc.sync.dma_start(xt1[:], xtv[t, 1])
                        xth = [xt0, xt1]
                        ps = psC.tile([P, 2, 2, NB], f32, tag="psy")
                        for q in range(2):
                            for h in range(2):
                                wblk = ws[:, h, q * P : (q + 1) * P]
                                for n in range(2):
                                    nc.tensor.matmul(
                                        ps[:, q, n],
                                        wblk,
                                        xth[h][:, n * NB : (n + 1) * NB],
                                        start=(h == 0),
                                        stop=(h == 1),
                                    )
                        # ELU(v+b) = min(exp(v+b)-1, relu(v+b)), b per-partition
                        for q in range(2):
                            bcol = aff_sb[:, 2 + q : 3 + q]
                            e = cp.tile([P, 2 * NB], bf16, tag=f"e{q}")
                            nc.scalar.activation(
                                e[:], ps[:, q], AF.Exp, bias=bcol
                            )
                            r = cp.tile([P, 2 * NB], bf16, tag=f"r{q}")
                            if q == 0:
                                nc.scalar.activation(
                                    r[:], ps[:, q], AF.Relu, bias=bcol
                                )
                            else:
                                nc.vector.tensor_scalar(
                                    r[:], ps[:, q], bcol, 0.0, OP.add, OP.max
                                )
                            yo = cp.tile([P, 2 * NB], bf16, tag=f"yo{q}")
                            nc.vector.scalar_tensor_tensor(
                                yo[:], e[:], 1.0, r[:], OP.subtract, OP.min
                            )
                            nc.sync.dma_start(ytv[t, q], yo[:])
    nc.compile()
    return nc


@functools.lru_cache(maxsize=4)
def _built_a(repeat=1):
    return build_a(repeat=repeat)


@functools.lru_cache(maxsize=4)
def _built_c(repeat=1):
    return build_c(repeat=repeat)


def _pjrt_fn(nc, ncores=NCORES):
    """Compile a bass module into a jitted 8-core shard_map callable.
    Returns (fn, in_names, out_names, out_avals, mesh)."""
    import jax
    from jax.experimental.shard_map import shard_map
    from jax.sharding import Mesh, PartitionSpec

    from concourse import mybir
    from concourse.bass2jax import (
        _bass_exec_p,
        install_neuronx_cc_hook,
        partition_id_tensor,
    )

    install_neuronx_cc_hook()
    partition_name = nc.partition_id_tensor.name if nc.partition_id_tensor else None
    in_names, out_names, out_avals = [], [], []
    for alloc in nc.m.functions[0].allocations:
        if not isinstance(alloc, mybir.MemoryLocationSet):
            continue
        name = alloc.memorylocations[0].name
        if alloc.kind == "ExternalInput":
            if name != partition_name:
                in_names.append(name)
        elif alloc.kind == "ExternalOutput":
            out_names.append(name)
            out_avals.append(
                jax.core.ShapedArray(
                    tuple(alloc.tensor_shape), mybir.dt.np(alloc.dtype)
                )
            )
    n_params = len(in_names)
    all_in_names = list(in_names) + list(out_names)
    if partition_name is not None:
        all_in_names.append(partition_name)

    def _body(*args):
        operands = list(args)
        if partition_name is not None:
            operands.append(partition_id_tensor())
        outs = _bass_exec_p.bind(
            *operands,
            out_avals=tuple(out_avals),
            in_names=tuple(all_in_names),
            out_names=tuple(out_names),
            lowering_input_output_aliases=(),
            sim_require_finite=True,
            sim_require_nnan=True,
            nc=nc,
        )
        return tuple(outs)

    devices = jax.devices()[:ncores]
    mesh = Mesh(np.asarray(devices), ("core",))
    spec = PartitionSpec("core")
    fn = jax.jit(
        shard_map(
            _body,
            mesh=mesh,
            in_specs=(spec,) * (n_params + len(out_names)),
            out_specs=(spec,) * len(out_names),
            check_rep=False,
        ),
        keep_unused=True,
    )
    return fn, in_names, out_names, out_avals, mesh


def _sharding():
    import jax
    from jax.sharding import Mesh, NamedSharding, PartitionSpec

    devices = jax.devices()[:NCORES]
    mesh = Mesh(np.asarray(devices), ("core",))
    return NamedSharding(mesh, PartitionSpec("core"))


def _zeros_for(out_avals):
    return [
        np.zeros((NCORES * av.shape[0], *av.shape[1:]), av.dtype) for av in out_avals
    ]


def kernel(x, gamma, beta, W):
    import jax
    import jax.numpy as jnp

    gamma = np.asarray(gamma, dtype=np.float64)
    beta = np.asarray(beta, dtype=np.float64)
    W = np.asarray(W, dtype=np.float32)
    assert np.asarray(x).shape == (N_TOTAL, F)

    cpu = jax.devices("cpu")[0]
    with jax.default_device(cpu):
        x_bf = np.asarray(jnp.asarray(np.asarray(x)).astype(jnp.bfloat16))

    sharding = _sharding()

    # ---- NEFF A: on-chip transpose + staging + partial stats
    nc_a = _built_a()
    fn_a, in_a, out_a, av_a, _ = _pjrt_fn(nc_a)
    ident = np.concatenate([np.eye(P, dtype=x_bf.dtype)] * NCORES, axis=0)
    host_a = {"x": x_bf, "ident": ident}
    args_a = [jax.device_put(host_a[nm], sharding) for nm in in_a]
    outs_a = fn_a(*args_a, *[jax.device_put(z, sharding) for z in _zeros_for(av_a)])
    outs_a = dict(zip(out_a, outs_a))

    # ---- host: reduce the 8 partial stat tiles (16 KB), finalize scale/shift
    st_host = np.asarray(outs_a["st"]).astype(np.float64)  # [8*128, 4]
    st_sum = st_host.reshape(NCORES, P, 4).sum(axis=0)  # [128, 4]
    mean = st_sum[:, 0:2] / N_TOTAL  # [128, 2] (h columns)
    var = st_sum[:, 2:4] / N_TOTAL - mean**2
    g_cols = np.stack([gamma[0:P], gamma[P:F]], axis=1)
    b_cols = np.stack([beta[0:P], beta[P:F]], axis=1)
    s_cols = g_cols / np.sqrt(var + EPS)
    t_cols = b_cols - mean * s_cols
    # linear bias row b = t @ W.T, split into f_out halves (per-partition on C)
    t_vec = np.concatenate([t_cols[:, 0], t_cols[:, 1]])
    b_row = t_vec @ W.astype(np.float64).T
    bq_cols = np.stack([b_row[0:P], b_row[P:F]], axis=1)
    aff = np.concatenate([s_cols, bq_cols], axis=1).astype(np.float32)  # [128,4]

    # ---- NEFF C: matmul + ELU (staging stays on device)
    nc_c = _built_c()
    fn_c, in_c, out_c, av_c, _ = _pjrt_fn(nc_c)
    host_c = {
        "wt": np.concatenate([np.ascontiguousarray(W.T)] * NCORES, axis=0),
        "aff": np.concatenate([aff] * NCORES, axis=0),
    }
    args_c = []
    for nm in in_c:
        if nm == "xt":
            args_c.append(outs_a["xt"])
        else:
            args_c.append(jax.device_put(host_c[nm], sharding))
    outs_c = fn_c(*args_c, *[jax.device_put(z, sharding) for z in _zeros_for(av_c)])
    y_bf = np.asarray(outs_c[out_c.index("yt")])
    with jax.default_device(cpu):
        # yt blocked [core, t, q, p, j, c] -> y[row t*1024+c*8+j, fout q*128+p]
        yt6 = jnp.asarray(y_bf).reshape(NCORES, T, 2, P, RT, P)
        y = np.asarray(
            jnp.transpose(yt6, (0, 1, 5, 4, 2, 3))
            .astype(jnp.float32)
            .reshape(N_TOTAL, F)
        )
    return np.ascontiguousarray(y)


if __name__ == "__main__":
    nca = build_a()
    ncc = build_c()
    print("built OK")
